# revision 7
# baseline (speedup 1.0000x reference)
"""Trainium2 Bass kernel for GQA causal attention (nn_Attention_37203006718300).

Reference computation (B=2, N=1024, D=2048, H=32 Q-heads, J=8 KV-heads, hd=64):
    q/k/v projections -> RoPE(q,k) -> causal GQA attention -> out @ wo

Distribution (8 NeuronCores, one TRN2 chip):
  Phase 1+2 (projections + attention): tensor-parallel over heads. Core c
    owns Q heads [4c, 4c+4) and KV head c: wq column-shard [2048,256],
    wk/wv column-shards [2048,64]. Every core holds full x (transposed).
  Handoff: AllToAll moves per-head attention outputs to per-token-block
    ownership (core j gets flat tokens [256j, 256j+256)).
  Phase 3 (output projection): token-parallel. Each core computes its
    256-token block against the full wo (bf16). Host concatenates blocks.

Layout: activations kept transposed ([feature, token]) throughout phases
1-2 so every matmul contraction sits on the partition axis. scores are
computed transposed ([key, query]); softmax denominators come free from a
ones-column appended to V; normalization uses a K=1 outer-product matmul
to broadcast 1/den across partitions.

All matmuls run in float32r (full PE rate at free-dim>=256) except the
output projection which runs in bf16. Causal masking only touches the
128x128 diagonal tiles (single additive mask tile); strictly-upper tiles
are never computed.
"""

import math

import numpy as np

# ---------------------------------------------------------------- constants
B = 2
N = 1024
D = 2048
H = 32
J = 8
HD = 64
ROPE_THETA = 10000.0
N_CORES = 8

T = B * N  # 2048 flat tokens
QH = H // N_CORES  # 4 Q heads per core
QCOLS = QH * HD  # 256
KVC = HD  # 64 kv cols per core
NDC = D // 128  # 16 contraction chunks of 128
TB = 512  # token block for phase 1 (4 blocks)
NTB = T // TB
QB = 512  # query block for attention
KC = 128  # key chunk
TOK_BLK = T // N_CORES  # 256 tokens per core in phase 3
MASK_VAL = -10000.0

_cache: dict = {}


# ---------------------------------------------------------------- program
def _build_program():
    import concourse.bacc as bacc
    import concourse.mybir as mybir
    import concourse.tile as tile

    dt = mybir.dt
    f32 = dt.float32
    f32r = dt.float32r
    bf16 = dt.bfloat16

    nc = bacc.Bacc(
        "TRN2", target_bir_lowering=False, debug=False, num_devices=N_CORES
    )

    # -------- DRAM I/O (per-core values supplied via in_maps)
    xT = nc.dram_tensor("xT", [D, T], f32r, kind="ExternalInput").ap()
    wq = nc.dram_tensor("wq", [D, QCOLS], f32r, kind="ExternalInput").ap()
    wkv = nc.dram_tensor("wkv", [D, 2 * KVC], f32r, kind="ExternalInput").ap()
    wo = nc.dram_tensor("wo", [D, D], bf16, kind="ExternalInput").ap()
    cosq = nc.dram_tensor("cosq", [128, N], f32, kind="ExternalInput").ap()
    sinq = nc.dram_tensor("sinq", [128, N], f32, kind="ExternalInput").ap()
    psw = nc.dram_tensor("psw", [128, 128], f32r, kind="ExternalInput").ap()
    id2 = nc.dram_tensor("id2", [128, 64], f32r, kind="ExternalInput").ap()
    mask = nc.dram_tensor("mask", [128, 128], f32, kind="ExternalInput").ap()
    ones2 = nc.dram_tensor("ones2", [65, 64], f32r, kind="ExternalInput").ap()
    col1 = nc.dram_tensor("col1", [128, 1], f32r, kind="ExternalInput").ap()
    out_ext = nc.dram_tensor("out", [TOK_BLK, D], f32, kind="ExternalOutput").ap()

    with tile.TileContext(nc) as tc:
        with (
            tc.tile_pool(name="const", bufs=1) as constp,
            tc.tile_pool(name="persist", bufs=1) as persist,
            tc.tile_pool(name="xt", bufs=4) as xtp,
            tc.tile_pool(name="work", bufs=2) as work,
            tc.tile_pool(name="expp", bufs=4) as expp,
            tc.tile_pool(name="wop", bufs=3) as wop,
            tc.tile_pool(name="psproj", bufs=2, space="PSUM") as psproj,
            tc.tile_pool(name="psmm", bufs=3, space="PSUM") as psmm,
            tc.tile_pool(name="psacc", bufs=2, space="PSUM") as psacc,
            tc.tile_pool(name="pst", bufs=1, space="PSUM") as pst,
            tc.tile_pool(name="dram", bufs=1, space="DRAM") as dram,
        ):
            # -------- constants / tables
            cos_sb = constp.tile([128, N], f32, tag="cos")
            nc.sync.dma_start(cos_sb[:], cosq[:])
            sin_sb = constp.tile([128, N], f32, tag="sin")
            nc.sync.dma_start(sin_sb[:], sinq[:])
            psw_sb = constp.tile([128, 128], f32r, tag="psw")
            nc.sync.dma_start(psw_sb[:], psw[:])
            id2_sb = constp.tile([128, 64], f32r, tag="id2")
            nc.sync.dma_start(id2_sb[:], id2[:])
            mask_sb = constp.tile([128, 128], f32, tag="mask")
            nc.sync.dma_start(mask_sb[:], mask[:])
            ones2_sb = constp.tile([65, 64], f32r, tag="ones2")
            nc.sync.dma_start(ones2_sb[:], ones2[:])

            # -------- weights (resident)
            wq_sb = constp.tile([128, NDC, QCOLS], f32r, tag="wq")
            nc.sync.dma_start(wq_sb[:], wq.rearrange("(a p) c -> p a c", p=128))
            wkv_sb = constp.tile([128, NDC, 2 * KVC], f32r, tag="wkv")
            nc.sync.dma_start(wkv_sb[:], wkv.rearrange("(a p) c -> p a c", p=128))

            xTr = xT.rearrange("(a p) t -> p a t", p=128)  # [128, 16, T]

            # -------- persistent activation tiles
            # q pairs: rows = 2 heads x 64; RoPE'd
            qrot = [
                persist.tile([128, T], f32r, tag=f"qrot{p}", name=f"qrot{p}")
                for p in range(2)
            ]
            qodd = [
                persist.tile([64, T], f32r, tag=f"qodd{p}", name=f"qodd{p}")
                for p in range(2)
            ]
            ktR = persist.tile([64, T], f32r, tag="ktR", name="ktR")
            # V (natural layout) per 128-token chunk, with ones column at 64
            vext = [
                persist.tile([128, 65], f32r, tag=f"vext{g}", name=f"vext{g}")
                for g in range(T // KC)
            ]
            for g in range(T // KC):
                nc.sync.dma_start(vext[g][:, 64:65], col1[:])

            # ================= phase 1: projections + RoPE =================
            for tb in range(NTB):
                ts, te = tb * TB, (tb + 1) * TB
                pos0 = (tb % (N // TB)) * TB  # position offset within batch
                xt_q = []
                for qtr in range(4):
                    xt_sb = xtp.tile([128, NDC // 4, TB], f32r, tag="xt", name="xt")
                    nc.sync.dma_start(
                        xt_sb[:], xTr[:, qtr * (NDC // 4) : (qtr + 1) * (NDC // 4), ts:te]
                    )
                    xt_q.append(xt_sb)

                def xt_chunk(a):
                    return xt_q[a // (NDC // 4)][:, a % (NDC // 4), :]

                cos_blk = cos_sb[:, pos0 : pos0 + TB]
                sin_blk = sin_sb[:, pos0 : pos0 + TB]

                # --- q pairs
                for p in range(2):
                    ps_q = psproj.tile([128, TB], f32, tag="proj", name="ps_q")
                    for a in range(NDC):
                        nc.tensor.matmul(
                            ps_q[:],
                            wq_sb[:, a, 128 * p : 128 * (p + 1)],
                            xt_chunk(a),
                            start=(a == 0),
                            stop=(a == NDC - 1),
                        )
                    qraw = work.tile([128, TB], f32r, tag="qraw", name="qraw")
                    nc.vector.tensor_copy(qraw[:], ps_q[:])
                    # pair-swap via PE
                    ps_sw = psmm.tile([128, TB], f32, tag="mm", name="ps_sw")
                    nc.tensor.matmul(ps_sw[:], psw_sb[:], qraw[:])
                    # rope: qrot = cos*q + sin_signed*swap(q)
                    t1 = work.tile([128, TB], f32, tag="t1", name="t1")
                    nc.vector.tensor_mul(t1[:], qraw[:], cos_blk)
                    t2 = work.tile([128, TB], f32, tag="t2", name="t2")
                    nc.vector.tensor_mul(t2[:], ps_sw[:], sin_blk)
                    nc.vector.tensor_add(qrot[p][:, ts:te], t1[:], t2[:])

                # --- kv
                ps_kv = psproj.tile([128, TB], f32, tag="proj", name="ps_kv")
                for a in range(NDC):
                    nc.tensor.matmul(
                        ps_kv[:],
                        wkv_sb[:, a, :],
                        xt_chunk(a),
                        start=(a == 0),
                        stop=(a == NDC - 1),
                    )
                kvraw = work.tile([128, TB], f32r, tag="kvraw", name="kvraw")
                nc.vector.tensor_copy(kvraw[:], ps_kv[:])
                # k rope (rows 0:64)
                ps_swk = psmm.tile([64, TB], f32, tag="mm", name="ps_swk")
                nc.tensor.matmul(ps_swk[:], psw_sb[0:64, 0:64], kvraw[0:64, :])
                t1k = work.tile([64, TB], f32, tag="t1", name="t1k")
                nc.vector.tensor_mul(t1k[:], kvraw[0:64, :], cos_blk[0:64, :])
                t2k = work.tile([64, TB], f32, tag="t2", name="t2k")
                nc.vector.tensor_mul(t2k[:], ps_swk[:], sin_blk[0:64, :])
                nc.vector.tensor_add(ktR[:, ts:te], t1k[:], t2k[:])
                # v transpose to natural layout (rows 64:128 of kvraw)
                for s in range(TB // KC):
                    g = tb * (TB // KC) + s
                    ps_t = pst.tile([128, 64], f32r, tag="tp", name="ps_t")
                    nc.tensor.transpose(
                        ps_t[:],
                        kvraw[64:128, s * KC : (s + 1) * KC],
                        id2_sb[64:128, :],
                    )
                    nc.vector.tensor_copy(vext[g][:, 0:64], ps_t[:])

            # split odd heads to base-partition-0 tiles (SBUF->SBUF DMA)
            for p in range(2):
                nc.sync.dma_start(qodd[p][:], qrot[p][64:128, :])

            # ================= phase 2: attention =================
            a2a_in = dram.tile([N_CORES, TOK_BLK, TOK_BLK], bf16, name="a2a_in")
            a2a_out = dram.tile([N_CORES, TOK_BLK, TOK_BLK], bf16, name="a2a_out")

            for b in range(B):
                for h in range(QH):
                    p, par = h // 2, h % 2
                    qsrc = qrot[p][0:64, :] if par == 0 else qodd[p][:]
                    ps_os = []
                    for qb in range(N // QB):
                        qs = b * N + qb * QB  # flat token start of q block
                        ps_o = psacc.tile([65, QB], f32, tag="acc", name="ps_o")
                        ps_os.append(ps_o)
                        nkc = (qb + 1) * (QB // KC)
                        for kc in range(nkc):
                            m = kc - qb * (QB // KC)  # >=0 on diagonal band
                            n0 = 128 * max(0, m)
                            ks = b * N + kc * KC
                            ps_s = psmm.tile([128, QB], f32, tag="mm", name="ps_s")
                            nc.tensor.matmul(
                                ps_s[:, n0:QB],
                                ktR[:, ks : ks + KC],
                                qsrc[:, qs + n0 : qs + QB],
                            )
                            if m >= 0:
                                nc.vector.tensor_add(
                                    ps_s[:, n0 : n0 + 128],
                                    ps_s[:, n0 : n0 + 128],
                                    mask_sb[:],
                                )
                            ex = expp.tile([128, QB], f32r, tag="exp", name="ex")
                            nc.scalar.activation(
                                ex[:, n0:QB],
                                ps_s[:, n0:QB],
                                mybir.ActivationFunctionType.Exp,
                                scale=1.0 / math.sqrt(HD),
                            )
                            g = (b * N) // KC + kc
                            nc.tensor.matmul(
                                ps_o[:, n0:QB],
                                vext[g][:],
                                ex[:, n0:QB],
                                start=(kc == 0),
                                stop=(kc == nkc - 1),
                                skip_group_check=True,
                            )
                    # normalization: den sits in row 64 of ps_o
                    den_sb = work.tile([65, N], f32r, tag="den", name="den_sb")
                    outTn = work.tile([64, N], bf16, tag="outTn", name="outTn")
                    for qb in range(N // QB):
                        ps_o = ps_os[qb]
                        nc.vector.reciprocal(ps_o[64:65, :], ps_o[64:65, :])
                        nc.scalar.copy(
                            den_sb[64:65, qb * QB : (qb + 1) * QB], ps_o[64:65, :]
                        )
                        ps_b = psmm.tile([64, QB], f32, tag="mm", name="ps_b")
                        nc.tensor.matmul(
                            ps_b[:],
                            ones2_sb[64:65, :],
                            den_sb[64:65, qb * QB : (qb + 1) * QB],
                        )
                        rec_sb = work.tile([64, QB], f32, tag="rec", name="rec_sb")
                        nc.scalar.copy(rec_sb[:], ps_b[:])
                        nc.vector.tensor_mul(
                            outTn[:, qb * QB : (qb + 1) * QB], ps_o[0:64, :], rec_sb[:]
                        )
                    # scatter into a2a input: chunk j = token block, rows = head
                    nc.sync.dma_start(
                        a2a_in[
                            b * (N // TOK_BLK) : (b + 1) * (N // TOK_BLK),
                            h * 64 : (h + 1) * 64,
                            :,
                        ].rearrange("j p t -> p j t"),
                        outTn.rearrange("p (j t) -> p j t", t=TOK_BLK),
                    )

            nc.gpsimd.collective_compute(
                "AllToAll",
                mybir.AluOpType.bypass,
                replica_groups=[list(range(N_CORES))],
                ins=[a2a_in.opt()],
                outs=[a2a_out.opt()],
            )

            # ================= phase 3: output projection =================
            attn_sb = persist.tile([128, NDC, TOK_BLK], bf16, tag="attn", name="attn_sb")
            nc.sync.dma_start(
                attn_sb[:],
                a2a_out.rearrange("c q t -> (c q) t").rearrange(
                    "(a p) t -> p a t", p=128
                ),
            )
            wor = wo.rearrange("(a p) n -> p a n", p=128)  # [128, 16, D]
            for nb in range(D // 512):
                wo_h = []
                for hf in range(2):
                    wo_sb = wop.tile([128, NDC // 2, 512], bf16, tag="wo", name="wo_sb")
                    nc.sync.dma_start(
                        wo_sb[:],
                        wor[
                            :,
                            hf * (NDC // 2) : (hf + 1) * (NDC // 2),
                            nb * 512 : (nb + 1) * 512,
                        ],
                    )
                    wo_h.append(wo_sb)
                for mh in range(TOK_BLK // 128):
                    ps_f = psmm.tile([128, 512], f32, tag="mm", name="ps_f")
                    for a in range(NDC):
                        nc.tensor.matmul(
                            ps_f[:],
                            attn_sb[:, a, 128 * mh : 128 * (mh + 1)],
                            wo_h[a // (NDC // 2)][:, a % (NDC // 2), :],
                            start=(a == 0),
                            stop=(a == NDC - 1),
                        )
                    o_sb = work.tile([128, 512], f32, tag="osb", name="o_sb")
                    nc.scalar.copy(o_sb[:], ps_f[:])
                    nc.sync.dma_start(
                        out_ext[128 * mh : 128 * (mh + 1), nb * 512 : (nb + 1) * 512],
                        o_sb[:],
                    )

    nc.compile()
    return nc


# ---------------------------------------------------------------- host prep
def _make_tables():
    import ml_dtypes

    freq = ROPE_THETA ** (-(np.arange(HD // 2, dtype=np.float64) * (2.0 / HD)))
    phase = np.arange(N, dtype=np.float64)[:, None] * freq[None, :]  # [N, 32]
    c = np.cos(phase)  # [N, 32]
    s = np.sin(phase)
    # row d of [128, N]: freq index (d % 64) // 2 ; sin sign: -1 for even d
    didx = np.arange(128)
    fidx = (didx % HD) // 2
    sign = np.where(didx % 2 == 0, -1.0, 1.0)
    cosq = c[:, fidx].T.astype(np.float32)  # [128, N]
    sinq = (s[:, fidx].T * sign[:, None]).astype(np.float32)

    psw = np.zeros((128, 128), dtype=np.float32)
    psw[np.arange(128), np.arange(128) ^ 1] = 1.0

    id2 = np.zeros((128, 64), dtype=np.float32)
    id2[0:64] = np.eye(64, dtype=np.float32)
    id2[64:128] = np.eye(64, dtype=np.float32)

    jj, ii = np.meshgrid(np.arange(128), np.arange(128), indexing="ij")
    mask = np.where(jj <= ii, 0.0, MASK_VAL).astype(np.float32)

    ones2 = np.ones((65, 64), dtype=np.float32)
    return cosq, sinq, psw, id2, mask, ones2


def _prep_in_maps(x, wq, wk, wv, wo):
    import ml_dtypes

    xT = np.ascontiguousarray(x.reshape(T, D).T).astype(np.float32)
    wo_bf = wo.astype(ml_dtypes.bfloat16)
    cosq, sinq, psw, id2, mask, ones2 = _make_tables()
    in_maps = []
    for c in range(N_CORES):
        in_maps.append(
            {
                "xT": xT,
                "wq": np.ascontiguousarray(wq[:, c * QCOLS : (c + 1) * QCOLS]),
                "wkv": np.ascontiguousarray(
                    np.concatenate(
                        [
                            wk[:, c * KVC : (c + 1) * KVC],
                            wv[:, c * KVC : (c + 1) * KVC],
                        ],
                        axis=1,
                    )
                ),
                "wo": wo_bf,
                "cosq": cosq,
                "sinq": sinq,
                "psw": psw,
                "id2": id2,
                "mask": mask,
                "ones2": ones2,
                "col1": np.ones((128, 1), dtype=np.float32),
            }
        )
    return in_maps


# ---------------------------------------------------------------- runner
def _make_runner(nc):
    """Cached jit-once PJRT executor (mirrors run_bass_via_pjrt multi-core)."""
    import jax
    import jax.numpy as jnp
    import concourse.mybir as mybir
    from concourse import bass2jax
    from jax.experimental.shard_map import shard_map
    from jax.sharding import Mesh, PartitionSpec

    bass2jax.install_neuronx_cc_hook()

    partition_name = nc.partition_id_tensor.name if nc.partition_id_tensor else None
    in_names, out_names, out_avals = [], [], []
    for alloc in nc.m.functions[0].allocations:
        if not isinstance(alloc, mybir.MemoryLocationSet):
            continue
        name = alloc.memorylocations[0].name
        if alloc.kind == "ExternalInput":
            if name != partition_name:
                in_names.append(name)
        elif alloc.kind == "ExternalOutput":
            out_names.append(name)
            out_avals.append(
                jax.core.ShapedArray(
                    tuple(alloc.tensor_shape), mybir.dt.np(alloc.dtype)
                )
            )
    n_params = len(in_names)
    n_outs = len(out_names)
    all_in_names = in_names + out_names
    if partition_name is not None:
        all_in_names = all_in_names + [partition_name]

    def _body(*args):
        operands = list(args)
        if partition_name is not None:
            operands.append(bass2jax.partition_id_tensor())
        outs = bass2jax._bass_exec_p.bind(
            *operands,
            out_avals=tuple(out_avals),
            in_names=tuple(all_in_names),
            out_names=tuple(out_names),
            lowering_input_output_aliases=(),
            sim_require_finite=False,
            sim_require_nnan=False,
            nc=nc,
        )
        return tuple(outs)

    devices = jax.devices()[:N_CORES]
    mesh = Mesh(np.asarray(devices), ("core",))
    spec = PartitionSpec("core")
    sharded = jax.jit(
        shard_map(
            _body,
            mesh=mesh,
            in_specs=(spec,) * (n_params + n_outs),
            out_specs=(spec,) * n_outs,
            check_rep=False,
        ),
        keep_unused=True,
    )

    def prep_args(in_maps):
        concat_in = [
            np.concatenate([np.asarray(in_maps[c][k]) for c in range(N_CORES)], axis=0)
            for k in in_names
        ]
        concat_zeros = [
            np.zeros((N_CORES * a.shape[0], *a.shape[1:]), a.dtype) for a in out_avals
        ]
        from jax.sharding import NamedSharding

        sh = NamedSharding(mesh, spec)
        return [jax.device_put(a, sh) for a in concat_in + concat_zeros]

    def run(args):
        outs = sharded(*args)
        return {
            name: np.asarray(outs[i]).reshape(N_CORES, *out_avals[i].shape)
            for i, name in enumerate(out_names)
        }

    return prep_args, run, sharded


def _get_state():
    if "state" not in _cache:
        nc = _build_program()
        prep_args, run, sharded = _make_runner(nc)
        _cache["state"] = (nc, prep_args, run, sharded)
    return _cache["state"]


def kernel(x, wq, wk, wv, wo):
    _, prep_args, run, _ = _get_state()
    in_maps = _prep_in_maps(x, wq, wk, wv, wo)
    args = prep_args(in_maps)
    _cache["last_args"] = args
    outs = run(args)
    out = outs["out"].reshape(T, D).astype(np.float32).reshape(B, N, D)
    return out


def timed_exec(iters=10):
    """Re-execute the last kernel() invocation's device-resident args `iters`
    times; returns estimated per-execution wall seconds."""
    import time
    import jax

    _, _, _, sharded = _get_state()
    args = _cache["last_args"]
    r = sharded(*args)
    jax.block_until_ready(r)  # warm
    t0 = time.perf_counter()
    rs = [sharded(*args) for _ in range(iters)]
    jax.block_until_ready(rs[-1])
    t1 = time.perf_counter()
    return (t1 - t0) / iters
